# revision 1
# baseline (speedup 1.0000x reference)
"""Trainium2 Bass kernel for nn_InterpolatedCharacterEmbed.

Full (unsharded) inputs in, full output out. Internally:
  - host does all the cheap ragged index math (O(B*S) scalars),
  - valid (unmasked) rows are compacted and row-sharded across 8 cores,
  - each core computes out_row = A_row @ E  +  corr_row @ W2s  +  pos*v
    where A is the one-hot interpolation matrix (bf16 matmul),
    corr is the nonlinear silu remainder silu(-|pos*w1k|) (bf16 matmul over
    only the coordinate chunks that can be nonzero), and pos*v is the exact
    fp32 linear part of the MLP (v = relu(w1) @ w2, valid since b1 == 0 and
    pos >= 0 make relu(pos*w1k) = pos*relu(w1k)).
  - masked rows are never computed; the host scatters valid rows back into a
    zeros output.
"""

import math

import numpy as np

B, S, T, D, V = 16, 4096, 1024, 512, 256
N_CORES = 8
P = 128
X_CUT = 20.0  # |x| above which silu(-|x|) ~ 0 (< 4.2e-8)
TILES_PER_LOAD_CHUNK = 8  # columns of A^T / pos_bcast DMA'd per chunk
LAST = {}  # debug/profiling stash: last BassKernelResults


def _host_prep(text, mask, embed, w1, b1, w2, b2):
    al = mask.sum(1).astype(np.int64)  # [B] audio lengths (prefix mask)
    tlf = (text >= 0).sum(1).astype(np.float32)  # [B] text lengths
    i = np.arange(S, dtype=np.float32)[None, :]
    alf = al.astype(np.float32)[:, None]
    src = np.clip((i + 0.5) * tlf[:, None] / alf - 0.5, 0.0, tlf[:, None] - 1.0)
    lo = np.floor(src).astype(np.int64)
    hi = np.minimum(lo + 1, tlf.astype(np.int64)[:, None] - 1)
    w = (src - lo).astype(np.float32)
    tok_lo = np.take_along_axis(text, lo, axis=1).astype(np.int64)
    tok_hi = np.take_along_axis(text, hi, axis=1).astype(np.int64)
    pos = np.where(
        alf > 1.0, tlf[:, None] * i / np.maximum(alf - 1.0, 1.0), 0.0
    ).astype(np.float32)

    # flattened valid rows (s < al[b]); mask is a prefix of ones
    valid_b = np.repeat(np.arange(B, dtype=np.int64), al)
    valid_s = np.concatenate([np.arange(a, dtype=np.int64) for a in al])
    flat_idx = valid_b * S + valid_s  # row index into [B*S, D] output
    nv = len(flat_idx)

    g_tok_lo = tok_lo[valid_b, valid_s]
    g_tok_hi = tok_hi[valid_b, valid_s]
    g_w = w[valid_b, valid_s]
    g_pos = pos[valid_b, valid_s]

    rows_per_core = int(math.ceil(nv / N_CORES / P)) * P
    n_tiles = rows_per_core // P
    return dict(
        nv=nv,
        flat_idx=flat_idx,
        g_tok_lo=g_tok_lo,
        g_tok_hi=g_tok_hi,
        g_w=g_w,
        g_pos=g_pos,
        rows_per_core=rows_per_core,
        n_tiles=n_tiles,
    )


def _chunk_schedule(meta, w1s_abs, fast):
    """Per-tile-slot number of corr chunks, shared by all cores (SPMD).

    Each core sorts its tiles by need (descending); slot t runs
    max-over-cores of the t-th sorted need. Returns (sched, perms) where
    perms[c][t] = original tile index of core c assigned to slot t.
    """
    nv, r, n_tiles = meta["nv"], meta["rows_per_core"], meta["n_tiles"]
    g_pos = meta["g_pos"]
    needs = np.zeros((N_CORES, n_tiles), np.int64)
    if fast:
        for c in range(N_CORES):
            rows = g_pos[c * r : (c + 1) * r]
            rows = np.pad(rows, (0, r - len(rows)))
            pmin = rows.reshape(n_tiles, P).min(1)
            # coords with |w1|*pmin > X_CUT contribute ~0 for every row in
            # the tile; they form a prefix of the |w1|-descending order
            k0 = (w1s_abs[None, :] * pmin[:, None] > X_CUT).sum(1)
            needs[c] = 4 - k0 // P
    else:
        needs[:] = 4
    perms = [np.argsort(-needs[c], kind="stable") for c in range(N_CORES)]
    sorted_needs = np.stack([needs[c][perms[c]] for c in range(N_CORES)])
    sched = sorted_needs.max(0)
    return sched, perms


def _build_program(n_tiles, rows_per_core, sched, fast):
    import concourse.bass as bass
    import concourse.tile as tile
    from concourse import bacc, mybir

    r = rows_per_core
    f32 = mybir.dt.float32
    bf16 = mybir.dt.bfloat16
    sigmoid = mybir.ActivationFunctionType.Sigmoid
    mult = mybir.AluOpType.mult
    add = mybir.AluOpType.add

    nc = bacc.Bacc(
        "TRN2", target_bir_lowering=False, debug=False, enable_asserts=False
    )

    at0_d = nc.dram_tensor("at0", [P, r], bf16, kind="ExternalInput").ap()
    at1_d = nc.dram_tensor("at1", [P, r], bf16, kind="ExternalInput").ap()
    posb_d = nc.dram_tensor("posb", [1, r], f32, kind="ExternalInput").ap()
    pospp_d = nc.dram_tensor("pospp", [n_tiles, P], f32, kind="ExternalInput").ap()
    scl_d = nc.dram_tensor("scl", [4, P], f32, kind="ExternalInput").ap()
    bias_d = nc.dram_tensor("bias", [4, P], f32, kind="ExternalInput").ap()
    e_d = nc.dram_tensor("e", [2, P, D], bf16, kind="ExternalInput").ap()
    w2c_d = nc.dram_tensor("w2c", [4, P, D], bf16, kind="ExternalInput").ap()
    v_d = nc.dram_tensor("v", [1, D], f32, kind="ExternalInput").ap()
    out_d = nc.dram_tensor("out", [r, D], f32, kind="ExternalOutput").ap()

    ck = TILES_PER_LOAD_CHUNK * P
    n_load_chunks = (r + ck - 1) // ck

    with tile.TileContext(nc) as tc:
        with (
            tc.tile_pool(name="const", bufs=1) as cpool,
            tc.tile_pool(name="h", bufs=10) as hpool,
            tc.tile_pool(name="psum", bufs=8, space="PSUM") as ppool,
            tc.tile_pool(name="lin", bufs=6) as lpool,
            tc.tile_pool(name="out", bufs=6) as opool,
        ):
            e_sb = [cpool.tile([P, D], bf16, tag=f"e{j}", name=f"e{j}") for j in range(2)]
            for j in range(2):
                nc.sync.dma_start(e_sb[j][:], e_d[j])
            w2_sb = [cpool.tile([P, D], bf16, tag=f"w2_{j}", name=f"w2_{j}") for j in range(4)]
            for j in range(4):
                nc.sync.dma_start(w2_sb[j][:], w2c_d[j])
            v_sb = cpool.tile([P, D], f32, tag="v")
            nc.sync.dma_start(v_sb[:], v_d.broadcast_to([P, D]))
            scl_sb = cpool.tile([P, 4], f32, tag="scl")
            nc.sync.dma_start(scl_sb[:], scl_d.rearrange("a b -> b a"))
            bias_sb = cpool.tile([P, 4], f32, tag="bias")
            nc.sync.dma_start(bias_sb[:], bias_d.rearrange("a b -> b a"))
            pospp_sb = cpool.tile([P, n_tiles], f32, tag="pospp")
            nc.sync.dma_start(pospp_sb[:], pospp_d.rearrange("a b -> b a"))

            at_sb, posb_sb = [], []
            for li in range(n_load_chunks):
                w_cols = min(ck, r - li * ck)
                sl = slice(li * ck, li * ck + w_cols)
                a0 = cpool.tile([P, w_cols], bf16, tag=f"at0_{li}", name=f"at0_{li}")
                nc.sync.dma_start(a0[:], at0_d[:, sl])
                a1 = cpool.tile([P, w_cols], bf16, tag=f"at1_{li}", name=f"at1_{li}")
                nc.sync.dma_start(a1[:], at1_d[:, sl])
                pb = cpool.tile([P, w_cols], f32, tag=f"posb_{li}", name=f"posb_{li}")
                nc.sync.dma_start(pb[:], posb_d[:, sl].broadcast_to([P, w_cols]))
                at_sb.append((a0, a1))
                posb_sb.append(pb)

            for t in range(n_tiles):
                li, off = divmod(t * P, ck)
                a0, a1 = at_sb[li]
                pb = posb_sb[li]
                msl = slice(off, off + P)

                # silu(x) = x * sigmoid(x); we compute h' = p * sigmoid(s*p)
                # with the s factor pre-folded into the w2 chunk rows, so
                # corr = h' @ (diag(s) @ w2s) is exact up to bf16.
                h_tiles = []
                for ci in range(4 - int(sched[t]), 4):
                    sg = hpool.tile([P, P], f32, tag="sg", name=f"sg_{t}_{ci}")
                    if fast:
                        nc.scalar.activation(
                            sg[:], pb[:, msl], sigmoid, scale=scl_sb[:, ci : ci + 1]
                        )
                        xin = pb[:, msl]
                    else:
                        nc.scalar.activation(
                            sg[:],
                            pb[:, msl],
                            sigmoid,
                            scale=scl_sb[:, ci : ci + 1],
                            bias=bias_sb[:, ci : ci + 1],
                        )
                        x = hpool.tile([P, P], f32, tag="x", name=f"x_{t}_{ci}")
                        nc.vector.tensor_scalar(
                            x[:],
                            pb[:, msl],
                            scl_sb[:, ci : ci + 1],
                            bias_sb[:, ci : ci + 1],
                            mult,
                            add,
                        )
                        xin = x[:]
                    h = hpool.tile([P, P], bf16, tag="h", name=f"h_{t}_{ci}")
                    nc.gpsimd.tensor_tensor(h[:], xin, sg[:], mult)
                    h_tiles.append((ci, h))

                psum = ppool.tile([P, D], f32, tag="psum")
                nc.tensor.matmul(
                    psum[:], lhsT=a0[:, msl], rhs=e_sb[0][:], start=True, stop=False
                )
                nc.tensor.matmul(
                    psum[:],
                    lhsT=a1[:, msl],
                    rhs=e_sb[1][:],
                    start=False,
                    stop=not h_tiles,
                )
                for j, (ci, h) in enumerate(h_tiles):
                    nc.tensor.matmul(
                        psum[:],
                        lhsT=h[:],
                        rhs=w2_sb[ci][:],
                        start=False,
                        stop=j == len(h_tiles) - 1,
                    )

                lin = lpool.tile([P, D], f32, tag="lin")
                if t % 2 == 0:
                    nc.scalar.mul(lin[:], v_sb[:], pospp_sb[:, t : t + 1])
                else:
                    nc.gpsimd.tensor_scalar(
                        lin[:], v_sb[:], pospp_sb[:, t : t + 1], None, mult
                    )
                ot = opool.tile([P, D], f32, tag="out")
                nc.vector.tensor_add(ot[:], psum[:], lin[:])
                nc.sync.dma_start(out_d[t * P : (t + 1) * P, :], ot[:])

    nc.compile()
    return nc


def prepare(text, mask, max_seq_len, embed, w1, b1, w2, b2):
    """Host prep + program build. Returns (nc, in_maps, reassembly_state)."""
    import ml_dtypes

    bf = ml_dtypes.bfloat16
    text = np.asarray(text).astype(np.int64)
    mask = np.asarray(mask).astype(bool)
    embed = np.asarray(embed).astype(np.float32)
    w1 = np.asarray(w1).astype(np.float32)
    b1 = np.asarray(b1).astype(np.float32)
    w2 = np.asarray(w2).astype(np.float32)
    b2 = np.asarray(b2).astype(np.float32)

    meta = _host_prep(text, mask, embed, w1, b1, w2, b2)
    nv, r, n_tiles = meta["nv"], meta["rows_per_core"], meta["n_tiles"]

    fast = bool(np.all(b1 == 0.0) and np.all(meta["g_pos"] >= 0.0))

    # sorted-by-|w1| coordinate order for the suffix-chunk trick
    order = np.argsort(-np.abs(w1), kind="stable")
    w1s = w1[order]
    w2s = w2[order]
    if fast:
        scl = -np.abs(w1s).astype(np.float32)  # corr h' = p * sigmoid(scl*p)
        biases = np.zeros(D, np.float32)
        v = (
            np.maximum(w1, 0.0).astype(np.float64) @ w2.astype(np.float64)
        ).astype(np.float32)
        w2ship = (scl[:, None].astype(np.float64) * w2s.astype(np.float64)).astype(
            np.float32
        )
    else:
        scl = w1s.astype(np.float32)  # full h = x*sigmoid(x), x = scl*p + b1
        biases = b1[order].astype(np.float32)
        v = np.zeros(D, np.float32)
        w2ship = w2s

    sched, perms = _chunk_schedule(meta, np.abs(w1s), fast)

    # per-core inputs, tiles permuted so slot t has >= its scheduled chunks
    in_maps = []
    gidx_per_core = []
    g_tok_lo, g_tok_hi = meta["g_tok_lo"], meta["g_tok_hi"]
    g_w, g_pos = meta["g_w"], meta["g_pos"]
    for c in range(N_CORES):
        slot = np.repeat(perms[c] * P, P) + np.tile(np.arange(P), n_tiles)
        gidx = c * r + slot  # global valid-row index, may exceed nv (pad)
        ok = gidx < nv
        gi = np.where(ok, gidx, 0)
        tl_c = np.where(ok, g_tok_lo[gi], 0)
        th_c = np.where(ok, g_tok_hi[gi], 0)
        w_c = np.where(ok, g_w[gi], 0.0).astype(np.float32)
        omw_c = np.where(ok, 1.0 - g_w[gi], 0.0).astype(np.float32)
        pos_c = np.where(ok, g_pos[gi], 0.0).astype(np.float32)

        at = np.zeros((V, r), np.float32)
        cols = np.arange(r)
        np.add.at(at, (tl_c, cols), omw_c)
        np.add.at(at, (th_c, cols), w_c)
        at = at.reshape(2, P, r).astype(bf)

        in_maps.append(
            {
                "at0": np.ascontiguousarray(at[0]),
                "at1": np.ascontiguousarray(at[1]),
                "posb": pos_c[None, :],
                "pospp": np.ascontiguousarray(pos_c.reshape(n_tiles, P)),
                "scl": np.ascontiguousarray(scl.reshape(4, P)),
                "bias": np.ascontiguousarray(biases.reshape(4, P)),
                "e": np.ascontiguousarray(embed.reshape(2, P, D).astype(bf)),
                "w2c": np.ascontiguousarray(w2ship.reshape(4, P, D).astype(bf)),
                "v": v[None, :],
                "out": None,  # placeholder, removed below
            }
        )
        del in_maps[-1]["out"]
        gidx_per_core.append((gidx, ok))

    nc = _build_program(n_tiles, r, sched, fast)
    state = dict(
        meta=meta,
        gidx_per_core=gidx_per_core,
        fast=fast,
        b2=b2,
    )
    return nc, in_maps, state


def reassemble(results, state):
    meta = state["meta"]
    out_full = np.zeros((B * S, D), np.float32)
    flat_idx = meta["flat_idx"]
    for c in range(N_CORES):
        gidx, ok = state["gidx_per_core"][c]
        rows = results[c]["out"]
        out_full[flat_idx[gidx[ok]]] = rows[ok]
    if not state["fast"] and np.any(state["b2"] != 0.0):
        out_full[flat_idx] += state["b2"][None, :]
    return out_full.reshape(B, S, D)


def kernel(text, mask, max_seq_len, embed, w1, b1, w2, b2):
    nc, in_maps, state = prepare(text, mask, max_seq_len, embed, w1, b1, w2, b2)

    from concourse.bass_utils import run_bass_kernel_spmd

    kres = run_bass_kernel_spmd(nc, in_maps, list(range(N_CORES)))
    LAST["results"] = kres
    return reassemble(kres.results, state)

